# revision 17
# baseline (speedup 1.0000x reference)
# Bahdanau attention with coverage — TRN2 Bass kernel, 8-way batch-parallel.
#
#   eh    = enc @ Wh + bh                      (B,S,U)
#   sh    = dec @ Ws + bs                      (B,U)
#   ch    = cov @ Wc + bc                      (B,S,U)
#   feat  = tanh(eh + sh[:,None,:] + ch)
#   score = (feat @ Vw + bv)[..., 0]           (B,S)   (bv shifts cancel in softmax)
#   attn  = softmax(score)*mask / sum(...)
#   coverage = attn[...,None] + cov
#   context  = einsum('bs,bsd->bd', attn, enc)
#
# Sharding: batch dim across 8 cores (8 batches/core). Weights replicated.
#
# Per-core data path (b = local batch 0..7, S=2048 split into 16 s-tiles of
# 128, D=U=512 split into 4 chunks of 128):
#   * enc[b] is DMA-loaded with an fp32->bf16 cast (SWDGE) into natural
#     layout [128(s%128), 16(s_tile), 512(d)], then transposed on the PE
#     (64 transposes/batch) into encT [128(d%128), 4(d_chk), 16(s_tile), 128].
#   * feat psum tile [128(s),512(u)] accumulates 4 matmuls
#     encT_tile.T @ Wh_chunk plus one K=2 matmul whose two contraction rows
#     add cov[s]*Wc[u] (coverage rank-1) and 1*colbias_b[u]
#     (colbias_b = bh+bs+bc+dec[b]@Ws).
#   * ACT tanh -> feat bf16; one DVE scalar_tensor_tensor op computes
#     feat*Vw with fused row-sum -> score column [128,1] per s-tile.
#   * Per-batch masked softmax on [128,16] columns (partition_all_reduce for
#     cross-partition max/sum).
#   * context: 16 accumulating matmuls attn_col.T @ enc_nat_tile -> [1,512].
#   * attn columns transposed back to rows on PE; coverage = attn + cov.

import os
import numpy as np
import ml_dtypes

B, S, D, U = 64, 2048, 512, 512
NCORES = 8
BL = B // NCORES          # batches per core
ST = S // 128             # s-tiles per batch
KC = D // 128             # contraction chunks

LAST_EXEC_NS = None       # set when BASS_KERNEL_TRACE=1
_NC_CACHE = None


def _build_program():
    import concourse.bacc as bacc
    import concourse.tile as tile
    from concourse import mybir, bass_isa

    f32 = mybir.dt.float32
    bf16 = mybir.dt.bfloat16
    Tanh = mybir.ActivationFunctionType.Tanh
    Exp = mybir.ActivationFunctionType.Exp
    X = mybir.AxisListType.X
    MUL = mybir.AluOpType.mult
    RMAX = bass_isa.ReduceOp.max
    RADD = bass_isa.ReduceOp.add

    nc = bacc.Bacc("TRN2", target_bir_lowering=False, debug=False,
                   num_devices=NCORES)

    # ---- DRAM I/O (per-core shard shapes) ----
    enc = nc.dram_tensor("enc", (BL, S, D), f32, kind="ExternalInput").ap()
    cov16 = nc.dram_tensor("cov16", (2, BL * S), bf16, kind="ExternalInput").ap()
    wc_t = nc.dram_tensor("wc_t", (1, BL * U), bf16, kind="ExternalInput").ap()
    cov32 = nc.dram_tensor("cov32", (BL, S), f32, kind="ExternalInput").ap()
    mask = nc.dram_tensor("mask", (BL, S), f32, kind="ExternalInput").ap()
    decT = nc.dram_tensor("decT", (D, BL), bf16, kind="ExternalInput").ap()
    wh = nc.dram_tensor("wh", (D, U), bf16, kind="ExternalInput").ap()
    ws = nc.dram_tensor("ws", (D, U), bf16, kind="ExternalInput").ap()
    wc8 = nc.dram_tensor("wc8", (BL, U), bf16, kind="ExternalInput").ap()
    vw = nc.dram_tensor("vw", (1, U), bf16, kind="ExternalInput").ap()
    bias_sum = nc.dram_tensor("bias_sum", (1, U), f32, kind="ExternalInput").ap()
    eye16 = nc.dram_tensor("eye16", (128, 128), bf16, kind="ExternalInput").ap()
    eye32 = nc.dram_tensor("eye32", (128, 128), f32, kind="ExternalInput").ap()

    out_ctx = nc.dram_tensor("out_ctx", (BL, D), f32, kind="ExternalOutput").ap()
    out_attn = nc.dram_tensor("out_attn", (BL, S), f32, kind="ExternalOutput").ap()
    out_cov = nc.dram_tensor("out_cov", (BL, S), f32, kind="ExternalOutput").ap()

    with tile.TileContext(nc) as tc:
        with (
            tc.tile_pool(name="const", bufs=1) as const,
            tc.tile_pool(name="encpool", bufs=2) as encpool,
            tc.tile_pool(name="feats", bufs=3) as feats,
            tc.tile_pool(name="cols", bufs=2) as cols,
            tc.tile_pool(name="rows", bufs=2) as rows,
            tc.tile_pool(name="ps_feat", bufs=4, space="PSUM") as ps_feat,
            tc.tile_pool(name="ps_tr", bufs=2, space="PSUM") as ps_tr,
            tc.tile_pool(name="ps_ctx", bufs=1, space="PSUM") as ps_ctx,
            tc.tile_pool(name="ps_small", bufs=1, space="PSUM") as ps_small,
        ):
            # ---------- constants ----------
            wh_sb = const.tile([128, KC, U], bf16)
            nc.sync.dma_start(out=wh_sb, in_=wh.rearrange("(k p) u -> p k u", p=128))
            ws_sb = const.tile([128, KC, U], bf16)
            nc.sync.dma_start(out=ws_sb, in_=ws.rearrange("(k p) u -> p k u", p=128))
            decT_sb = const.tile([128, KC, BL], bf16)
            nc.sync.dma_start(out=decT_sb, in_=decT.rearrange("(k p) b -> p k b", p=128))
            covA = const.tile([2, BL * S], bf16)
            nc.sync.dma_start(out=covA, in_=cov16)
            eye16_sb = const.tile([128, 128], bf16)
            nc.sync.dma_start(out=eye16_sb, in_=eye16)
            eye32_sb = const.tile([128, 128], f32)
            nc.sync.dma_start(out=eye32_sb, in_=eye32)
            vw_row = const.tile([1, U], bf16)
            nc.sync.dma_start(out=vw_row, in_=vw)
            vw_bc = const.tile([128, U], bf16)
            nc.gpsimd.partition_broadcast(vw_bc, vw_row)
            bias_row = const.tile([1, U], f32)
            nc.sync.dma_start(out=bias_row, in_=bias_sum)
            bias_bc = const.tile([BL, U], f32)
            nc.gpsimd.partition_broadcast(bias_bc, bias_row)

            # ---------- colbias = dec @ Ws + (bh+bs+bc) ----------
            ps_sh = ps_feat.tile([BL, U], f32, tag="psfeat")
            for k in range(KC):
                nc.tensor.matmul(ps_sh, decT_sb[:, k, :], ws_sb[:, k, :],
                                 start=(k == 0), stop=(k == KC - 1))
            colbias = const.tile([BL, U], bf16)
            nc.vector.tensor_add(colbias, ps_sh, bias_bc)
            # wccb[0, b*U:] = Wc row, wccb[1, b*U:] = colbias row b
            wccb = const.tile([2, BL * U], bf16)
            nc.sync.dma_start(out=wccb[0:1, :], in_=wc_t)
            nc.sync.dma_start(out=wccb[1:2, :].rearrange("o (b u) -> o b u", b=BL),
                              in_=colbias)

            # ---------- per-batch main loop ----------
            for b in range(BL):
                # natural-layout bf16 enc (cast during DMA, SWDGE)
                enc_nat = encpool.tile([128, ST, D], bf16, tag="enc_nat")
                nc.gpsimd.dma_start(
                    out=enc_nat, in_=enc[b].rearrange("(t p) d -> p t d", p=128))
                # PE transposes -> encT [128(d%128), KC, ST, 128(s)]
                # 4 d-chunk transposes share one psum tile -> single wide copy
                encT = encpool.tile([128, KC, ST, 128], bf16, tag="encT")
                for t in range(ST):
                    pt = ps_tr.tile([128, KC, 128], bf16, tag="ptr")
                    for k in range(KC):
                        nc.tensor.transpose(
                            pt[:, k, :], enc_nat[:, t, k * 128:(k + 1) * 128],
                            eye16_sb)
                    nc.vector.tensor_copy(out=encT[:, :, t, :], in_=pt)

                # mask columns for this batch: [128, ST]
                mrow = rows.tile([16, 128], f32, tag="mrow")
                nc.sync.dma_start(out=mrow,
                                  in_=mask[b].rearrange("(t x) -> t x", x=128))
                pmt = ps_small.tile([128, 16], f32, tag="small")
                nc.tensor.transpose(pmt, mrow, eye32_sb[0:16, 0:16])
                maskT = cols.tile([128, ST], f32, tag="maskT")
                nc.vector.tensor_copy(out=maskT, in_=pmt)

                score_cols = cols.tile([128, ST], f32, tag="score")
                TG = 4  # s-tiles per group: 4 psum tiles share Wh chunk loads
                for tg in range(ST // TG):
                    pfs = [ps_feat.tile([128, U], f32, tag="psfeat",
                                        name=f"pf_{b}_{tg}_{i}")
                           for i in range(TG)]
                    for k in range(KC):
                        for tt in range(TG):
                            t = tg * TG + tt
                            nc.tensor.matmul(pfs[tt], encT[:, k, t, :],
                                             wh_sb[:, k, :],
                                             start=(k == 0), stop=False)
                    for tt in range(TG):
                        t = tg * TG + tt
                        pf = pfs[tt]
                        nc.tensor.matmul(
                            pf,
                            covA[:, b * S + t * 128: b * S + (t + 1) * 128],
                            wccb[:, b * U:(b + 1) * U],
                            start=False, stop=True)
                        feat = feats.tile([128, U], bf16, tag="feat")
                        nc.scalar.activation(out=feat, in_=pf, func=Tanh)
                        # prod = feat * Vw on gpsimd; row-sum on DVE
                        prod = feats.tile([128, U], bf16, tag="prod")
                        nc.gpsimd.tensor_mul(prod, feat, vw_bc)
                        nc.vector.reduce_sum(score_cols[:, t:t + 1], prod,
                                             axis=X)

                # ---------- masked softmax over s (columns layout) ----------
                rmax = cols.tile([128, 1], f32, tag="rmax")
                nc.vector.reduce_max(rmax, score_cols, axis=X)
                gmax = cols.tile([128, 1], f32, tag="gmax")
                nc.gpsimd.partition_all_reduce(gmax, rmax, channels=128,
                                               reduce_op=RMAX)
                negm = cols.tile([128, 1], f32, tag="negm")
                nc.scalar.mul(negm, gmax, -1.0)
                e_cols = cols.tile([128, ST], f32, tag="ecols")
                nc.scalar.activation(out=e_cols, in_=score_cols, func=Exp,
                                     bias=negm, scale=1.0)
                # emask = e*mask with fused row-sum
                emask = cols.tile([128, ST], f32, tag="emask")
                rsum = cols.tile([128, 1], f32, tag="rsum")
                nc.vector.scalar_tensor_tensor(
                    out=emask, in0=e_cols, scalar=1.0, in1=maskT,
                    op0=MUL, op1=MUL, accum_out=rsum)
                gsum = cols.tile([128, 1], f32, tag="gsum")
                nc.gpsimd.partition_all_reduce(gsum, rsum, channels=128,
                                               reduce_op=RADD)
                rcp = cols.tile([128, 1], f32, tag="rcp")
                nc.vector.reciprocal(rcp, gsum)
                attn_f = cols.tile([128, ST], f32, tag="attn_f")
                nc.vector.tensor_scalar_mul(attn_f, emask, rcp)
                attn_h = cols.tile([128, ST], bf16, tag="attn_h")
                nc.vector.tensor_scalar_mul(attn_h, emask, rcp)

                # ---------- context = attn @ enc ----------
                pctx = ps_ctx.tile([1, D], f32, tag="pctx")
                for t in range(ST):
                    nc.tensor.matmul(pctx, attn_h[:, t:t + 1], enc_nat[:, t, :],
                                     start=(t == 0), stop=(t == ST - 1))
                ctx_sb = rows.tile([1, D], f32, tag="ctx_sb")
                nc.scalar.copy(ctx_sb, pctx)
                nc.sync.dma_start(out=out_ctx[b:b + 1, :], in_=ctx_sb)

                # ---------- attn rows + coverage ----------
                pat = ps_small.tile([16, 128], f32, tag="small")
                nc.tensor.transpose(pat, attn_f, eye32_sb)
                attn_rows = rows.tile([16, 128], f32, tag="attn_rows")
                nc.scalar.copy(attn_rows, pat)
                nc.sync.dma_start(
                    out=out_attn[b].rearrange("(t x) -> t x", x=128), in_=attn_rows)
                cov_t = rows.tile([16, 128], f32, tag="cov_t")
                nc.sync.dma_start(out=cov_t,
                                  in_=cov32[b].rearrange("(t x) -> t x", x=128))
                cov_o = rows.tile([16, 128], f32, tag="cov_o")
                nc.vector.tensor_add(cov_o, pat, cov_t)
                nc.sync.dma_start(
                    out=out_cov[b].rearrange("(t x) -> t x", x=128), in_=cov_o)

    nc.finalize()
    return nc


def _get_program():
    global _NC_CACHE
    if _NC_CACHE is None:
        _NC_CACHE = _build_program()
    return _NC_CACHE


_LDW_PATCHED = False


def _maybe_patch_ldw_opt():
    # Optional experiment: let walrus optimize LDWEIGHTS (gated by env).
    global _LDW_PATCHED
    if _LDW_PATCHED or os.environ.get("BASS_LDW_OPT", "0") != "1":
        return
    import concourse.bass_utils as bu
    orig = bu.run_command

    def patched(argv, **kw):
        argv = ["--enable-ldw-opt=true" if a == "--enable-ldw-opt=false" else a
                for a in argv]
        return orig(argv, **kw)

    bu.run_command = patched
    _LDW_PATCHED = True


def kernel(dec_hidden, enc_output, enc_padding_mask, prev_coverage,
           Wh, bh, Ws, bs, Wc, bc, Vw, bv):
    global LAST_EXEC_NS
    _maybe_patch_ldw_opt()
    from concourse.bass_utils import run_bass_kernel_spmd

    nc = _get_program()
    bf = ml_dtypes.bfloat16

    dec_hidden = np.asarray(dec_hidden, dtype=np.float32)
    enc_output = np.asarray(enc_output, dtype=np.float32)
    enc_padding_mask = np.asarray(enc_padding_mask, dtype=np.float32)
    prev_coverage = np.asarray(prev_coverage, dtype=np.float32)

    cov2 = prev_coverage[..., 0]                      # (B, S)
    wh16 = np.asarray(Wh, np.float32).astype(bf)
    ws16 = np.asarray(Ws, np.float32).astype(bf)
    wc_row = np.asarray(Wc, np.float32).reshape(1, U)
    wc8 = np.ascontiguousarray(np.broadcast_to(wc_row, (BL, U))).astype(bf)
    vw_row = np.asarray(Vw, np.float32).reshape(1, U).astype(bf)
    bias_sum = (np.asarray(bh, np.float32).reshape(U)
                + np.asarray(bs, np.float32).reshape(U)
                + np.asarray(bc, np.float32).reshape(U)).reshape(1, U)
    eye16 = np.eye(128, dtype=np.float32).astype(bf)
    eye32 = np.eye(128, dtype=np.float32)

    wc_t = np.tile(wc_row.astype(bf), (1, BL))
    in_maps = []
    for c in range(NCORES):
        sl = slice(c * BL, (c + 1) * BL)
        covA = np.empty((2, BL * S), dtype=bf)
        covA[0] = cov2[sl].reshape(-1).astype(bf)
        covA[1] = np.ones((BL * S,), dtype=bf)
        in_maps.append({
            "enc": enc_output[sl],
            "cov16": covA,
            "wc_t": wc_t,
            "cov32": cov2[sl],
            "mask": enc_padding_mask[sl],
            "decT": dec_hidden[sl].T.astype(bf),
            "wh": wh16,
            "ws": ws16,
            "wc8": wc8,
            "vw": vw_row,
            "bias_sum": bias_sum,
            "eye16": eye16,
            "eye32": eye32,
        })

    trace = os.environ.get("BASS_KERNEL_TRACE", "0") == "1"
    res = run_bass_kernel_spmd(nc, in_maps, core_ids=list(range(NCORES)),
                               trace=trace)
    if trace:
        LAST_EXEC_NS = res.exec_time_ns

    ctx = np.concatenate([r["out_ctx"] for r in res.results], axis=0)
    attn = np.concatenate([r["out_attn"] for r in res.results], axis=0)
    cov = np.concatenate([r["out_cov"] for r in res.results], axis=0)[..., None]
    return ctx.astype(np.float32), attn.astype(np.float32), cov.astype(np.float32)


# revision 18
# speedup vs baseline: 1.6526x; 1.6526x over previous
# Bahdanau attention with coverage — TRN2 Bass kernel, 8-way batch-parallel.
#
#   eh    = enc @ Wh + bh                      (B,S,U)
#   sh    = dec @ Ws + bs                      (B,U)
#   ch    = cov @ Wc + bc                      (B,S,U)
#   feat  = tanh(eh + sh[:,None,:] + ch)
#   score = (feat @ Vw + bv)[..., 0]           (B,S)   (bv shifts cancel in softmax)
#   attn  = softmax(score)*mask / sum(...)
#   coverage = attn[...,None] + cov
#   context  = einsum('bs,bsd->bd', attn, enc)
#
# Sharding: batch dim across 8 cores (8 batches/core). Weights replicated.
#
# Per-core data path (b = local batch 0..7, S=2048 split into 16 s-tiles of
# 128, D=U=512 split into 4 chunks of 128):
#   * enc[b] is DMA-loaded with an fp32->bf16 cast (SWDGE) into natural
#     layout [128(s%128), 16(s_tile), 512(d)], then transposed on the PE
#     (64 transposes/batch) into encT [128(d%128), 4(d_chk), 16(s_tile), 128].
#   * feat psum tile [128(s),512(u)] accumulates 4 matmuls
#     encT_tile.T @ Wh_chunk plus one K=2 matmul whose two contraction rows
#     add cov[s]*Wc[u] (coverage rank-1) and 1*colbias_b[u]
#     (colbias_b = bh+bs+bc+dec[b]@Ws).
#   * ACT tanh -> feat bf16; one DVE scalar_tensor_tensor op computes
#     feat*Vw with fused row-sum -> score column [128,1] per s-tile.
#   * Per-batch masked softmax on [128,16] columns (partition_all_reduce for
#     cross-partition max/sum).
#   * context: 16 accumulating matmuls attn_col.T @ enc_nat_tile -> [1,512].
#   * attn columns transposed back to rows on PE; coverage = attn + cov.

import os
import numpy as np
import ml_dtypes

B, S, D, U = 64, 2048, 512, 512
NCORES = 8
BL = B // NCORES          # batches per core
ST = S // 128             # s-tiles per batch
KC = D // 128             # contraction chunks

LAST_EXEC_NS = None       # set when BASS_KERNEL_TRACE=1
_NC_CACHE = None


def _build_program():
    import concourse.bacc as bacc
    import concourse.tile as tile
    from concourse import mybir, bass_isa

    f32 = mybir.dt.float32
    bf16 = mybir.dt.bfloat16
    Tanh = mybir.ActivationFunctionType.Tanh
    Exp = mybir.ActivationFunctionType.Exp
    X = mybir.AxisListType.X
    MUL = mybir.AluOpType.mult
    RMAX = bass_isa.ReduceOp.max
    RADD = bass_isa.ReduceOp.add

    nc = bacc.Bacc("TRN2", target_bir_lowering=False, debug=False,
                   num_devices=NCORES)

    # ---- DRAM I/O (per-core shard shapes) ----
    enc = nc.dram_tensor("enc", (BL, S, D), f32, kind="ExternalInput").ap()
    cov16 = nc.dram_tensor("cov16", (2, BL * S), bf16, kind="ExternalInput").ap()
    wc_t = nc.dram_tensor("wc_t", (1, BL * U), bf16, kind="ExternalInput").ap()
    cov32 = nc.dram_tensor("cov32", (BL, S), f32, kind="ExternalInput").ap()
    mask = nc.dram_tensor("mask", (BL, S), f32, kind="ExternalInput").ap()
    decT = nc.dram_tensor("decT", (D, BL), bf16, kind="ExternalInput").ap()
    wh = nc.dram_tensor("wh", (D, U), bf16, kind="ExternalInput").ap()
    ws = nc.dram_tensor("ws", (D, U), bf16, kind="ExternalInput").ap()
    wc8 = nc.dram_tensor("wc8", (BL, U), bf16, kind="ExternalInput").ap()
    vw = nc.dram_tensor("vw", (1, U), bf16, kind="ExternalInput").ap()
    bias_sum = nc.dram_tensor("bias_sum", (1, U), f32, kind="ExternalInput").ap()
    eye16 = nc.dram_tensor("eye16", (128, 128), bf16, kind="ExternalInput").ap()
    eye32 = nc.dram_tensor("eye32", (128, 128), f32, kind="ExternalInput").ap()

    out_ctx = nc.dram_tensor("out_ctx", (BL, D), f32, kind="ExternalOutput").ap()
    out_attn = nc.dram_tensor("out_attn", (BL, S), f32, kind="ExternalOutput").ap()
    out_cov = nc.dram_tensor("out_cov", (BL, S), f32, kind="ExternalOutput").ap()

    with tile.TileContext(nc) as tc:
        with (
            tc.tile_pool(name="const", bufs=1) as const,
            tc.tile_pool(name="encpool", bufs=2) as encpool,
            tc.tile_pool(name="feats", bufs=3) as feats,
            tc.tile_pool(name="cols", bufs=2) as cols,
            tc.tile_pool(name="rows", bufs=2) as rows,
            tc.tile_pool(name="ps_feat", bufs=4, space="PSUM") as ps_feat,
            tc.tile_pool(name="ps_tr", bufs=2, space="PSUM") as ps_tr,
            tc.tile_pool(name="ps_ctx", bufs=1, space="PSUM") as ps_ctx,
            tc.tile_pool(name="ps_small", bufs=1, space="PSUM") as ps_small,
        ):
            # ---------- constants ----------
            wh_sb = const.tile([128, KC, U], bf16)
            nc.sync.dma_start(out=wh_sb, in_=wh.rearrange("(k p) u -> p k u", p=128))
            ws_sb = const.tile([128, KC, U], bf16)
            nc.sync.dma_start(out=ws_sb, in_=ws.rearrange("(k p) u -> p k u", p=128))
            decT_sb = const.tile([128, KC, BL], bf16)
            nc.sync.dma_start(out=decT_sb, in_=decT.rearrange("(k p) b -> p k b", p=128))
            covA = const.tile([2, BL * S], bf16)
            nc.sync.dma_start(out=covA, in_=cov16)
            eye16_sb = const.tile([128, 128], bf16)
            nc.sync.dma_start(out=eye16_sb, in_=eye16)
            eye32_sb = const.tile([128, 128], f32)
            nc.sync.dma_start(out=eye32_sb, in_=eye32)
            vw_row = const.tile([1, U], bf16)
            nc.sync.dma_start(out=vw_row, in_=vw)
            vw_bc = const.tile([128, U], bf16)
            nc.gpsimd.partition_broadcast(vw_bc, vw_row)
            bias_row = const.tile([1, U], f32)
            nc.sync.dma_start(out=bias_row, in_=bias_sum)
            bias_bc = const.tile([BL, U], f32)
            nc.gpsimd.partition_broadcast(bias_bc, bias_row)

            # ---------- colbias = dec @ Ws + (bh+bs+bc) ----------
            ps_sh = ps_feat.tile([BL, U], f32, tag="psfeat")
            for k in range(KC):
                nc.tensor.matmul(ps_sh, decT_sb[:, k, :], ws_sb[:, k, :],
                                 start=(k == 0), stop=(k == KC - 1))
            colbias = const.tile([BL, U], bf16)
            nc.vector.tensor_add(colbias, ps_sh, bias_bc)
            # wccb[0, b*U:] = Wc row, wccb[1, b*U:] = colbias row b
            wccb = const.tile([2, BL * U], bf16)
            nc.sync.dma_start(out=wccb[0:1, :], in_=wc_t)
            nc.sync.dma_start(out=wccb[1:2, :].rearrange("o (b u) -> o b u", b=BL),
                              in_=colbias)

            # ---------- per-batch main loop ----------
            for b in range(BL):
                # natural-layout bf16 enc (cast during DMA, SWDGE)
                enc_nat = encpool.tile([128, ST, D], bf16, tag="enc_nat")
                nc.gpsimd.dma_start(
                    out=enc_nat, in_=enc[b].rearrange("(t p) d -> p t d", p=128))
                # PE transposes -> encT [128(d%128), KC, ST, 128(s)]
                # 4 d-chunk transposes share one psum tile -> single wide copy
                encT = encpool.tile([128, KC, ST, 128], bf16, tag="encT")
                for t in range(ST):
                    pt = ps_tr.tile([128, KC, 128], bf16, tag="ptr")
                    for k in range(KC):
                        nc.tensor.transpose(
                            pt[:, k, :], enc_nat[:, t, k * 128:(k + 1) * 128],
                            eye16_sb)
                    nc.vector.tensor_copy(out=encT[:, :, t, :], in_=pt)

                # mask columns for this batch: [128, ST]
                mrow = rows.tile([16, 128], f32, tag="mrow")
                nc.sync.dma_start(out=mrow,
                                  in_=mask[b].rearrange("(t x) -> t x", x=128))
                pmt = ps_small.tile([128, 16], f32, tag="small")
                nc.tensor.transpose(pmt, mrow, eye32_sb[0:16, 0:16])
                maskT = cols.tile([128, ST], f32, tag="maskT")
                nc.vector.tensor_copy(out=maskT, in_=pmt)

                score_cols = cols.tile([128, ST], f32, tag="score")
                TG = 4  # s-tiles per group: 4 psum tiles share Wh chunk loads
                for tg in range(ST // TG):
                    pfs = [ps_feat.tile([128, U], f32, tag="psfeat",
                                        name=f"pf_{b}_{tg}_{i}")
                           for i in range(TG)]
                    for k in range(KC):
                        for tt in range(TG):
                            t = tg * TG + tt
                            nc.tensor.matmul(pfs[tt], encT[:, k, t, :],
                                             wh_sb[:, k, :],
                                             start=(k == 0), stop=False)
                    for tt in range(TG):
                        t = tg * TG + tt
                        pf = pfs[tt]
                        nc.tensor.matmul(
                            pf,
                            covA[:, b * S + t * 128: b * S + (t + 1) * 128],
                            wccb[:, b * U:(b + 1) * U],
                            start=False, stop=True)
                        feat = feats.tile([128, U], bf16, tag="feat")
                        nc.scalar.activation(out=feat, in_=pf, func=Tanh)
                        # prod = feat * Vw  (row-sum fused into accum_out, DVE)
                        prod = feats.tile([128, U], bf16, tag="prod")
                        nc.vector.scalar_tensor_tensor(
                            out=prod, in0=feat, scalar=1.0, in1=vw_bc,
                            op0=MUL, op1=MUL,
                            accum_out=score_cols[:, t:t + 1])

                # ---------- masked softmax over s (columns layout) ----------
                rmax = cols.tile([128, 1], f32, tag="rmax")
                nc.vector.reduce_max(rmax, score_cols, axis=X)
                gmax = cols.tile([128, 1], f32, tag="gmax")
                nc.gpsimd.partition_all_reduce(gmax, rmax, channels=128,
                                               reduce_op=RMAX)
                negm = cols.tile([128, 1], f32, tag="negm")
                nc.scalar.mul(negm, gmax, -1.0)
                e_cols = cols.tile([128, ST], f32, tag="ecols")
                nc.scalar.activation(out=e_cols, in_=score_cols, func=Exp,
                                     bias=negm, scale=1.0)
                # emask = e*mask with fused row-sum
                emask = cols.tile([128, ST], f32, tag="emask")
                rsum = cols.tile([128, 1], f32, tag="rsum")
                nc.vector.scalar_tensor_tensor(
                    out=emask, in0=e_cols, scalar=1.0, in1=maskT,
                    op0=MUL, op1=MUL, accum_out=rsum)
                gsum = cols.tile([128, 1], f32, tag="gsum")
                nc.gpsimd.partition_all_reduce(gsum, rsum, channels=128,
                                               reduce_op=RADD)
                rcp = cols.tile([128, 1], f32, tag="rcp")
                nc.vector.reciprocal(rcp, gsum)
                attn_f = cols.tile([128, ST], f32, tag="attn_f")
                nc.vector.tensor_scalar_mul(attn_f, emask, rcp)
                attn_h = cols.tile([128, ST], bf16, tag="attn_h")
                nc.vector.tensor_scalar_mul(attn_h, emask, rcp)

                # ---------- context = attn @ enc ----------
                pctx = ps_ctx.tile([1, D], f32, tag="pctx")
                for t in range(ST):
                    nc.tensor.matmul(pctx, attn_h[:, t:t + 1], enc_nat[:, t, :],
                                     start=(t == 0), stop=(t == ST - 1))
                ctx_sb = rows.tile([1, D], f32, tag="ctx_sb")
                nc.scalar.copy(ctx_sb, pctx)
                nc.sync.dma_start(out=out_ctx[b:b + 1, :], in_=ctx_sb)

                # ---------- attn rows + coverage ----------
                pat = ps_small.tile([16, 128], f32, tag="small")
                nc.tensor.transpose(pat, attn_f, eye32_sb)
                attn_rows = rows.tile([16, 128], f32, tag="attn_rows")
                nc.scalar.copy(attn_rows, pat)
                nc.sync.dma_start(
                    out=out_attn[b].rearrange("(t x) -> t x", x=128), in_=attn_rows)
                cov_t = rows.tile([16, 128], f32, tag="cov_t")
                nc.sync.dma_start(out=cov_t,
                                  in_=cov32[b].rearrange("(t x) -> t x", x=128))
                cov_o = rows.tile([16, 128], f32, tag="cov_o")
                nc.vector.tensor_add(cov_o, pat, cov_t)
                nc.sync.dma_start(
                    out=out_cov[b].rearrange("(t x) -> t x", x=128), in_=cov_o)

    nc.finalize()
    return nc


def _get_program():
    global _NC_CACHE
    if _NC_CACHE is None:
        _NC_CACHE = _build_program()
    return _NC_CACHE


_LDW_PATCHED = False


def _maybe_patch_ldw_opt():
    # Optional experiment: let walrus optimize LDWEIGHTS (gated by env).
    global _LDW_PATCHED
    if _LDW_PATCHED or os.environ.get("BASS_LDW_OPT", "0") != "1":
        return
    import concourse.bass_utils as bu
    orig = bu.run_command

    def patched(argv, **kw):
        argv = ["--enable-ldw-opt=true" if a == "--enable-ldw-opt=false" else a
                for a in argv]
        return orig(argv, **kw)

    bu.run_command = patched
    _LDW_PATCHED = True


def kernel(dec_hidden, enc_output, enc_padding_mask, prev_coverage,
           Wh, bh, Ws, bs, Wc, bc, Vw, bv):
    global LAST_EXEC_NS
    _maybe_patch_ldw_opt()
    from concourse.bass_utils import run_bass_kernel_spmd

    nc = _get_program()
    bf = ml_dtypes.bfloat16

    dec_hidden = np.asarray(dec_hidden, dtype=np.float32)
    enc_output = np.asarray(enc_output, dtype=np.float32)
    enc_padding_mask = np.asarray(enc_padding_mask, dtype=np.float32)
    prev_coverage = np.asarray(prev_coverage, dtype=np.float32)

    cov2 = prev_coverage[..., 0]                      # (B, S)
    wh16 = np.asarray(Wh, np.float32).astype(bf)
    ws16 = np.asarray(Ws, np.float32).astype(bf)
    wc_row = np.asarray(Wc, np.float32).reshape(1, U)
    wc8 = np.ascontiguousarray(np.broadcast_to(wc_row, (BL, U))).astype(bf)
    vw_row = np.asarray(Vw, np.float32).reshape(1, U).astype(bf)
    bias_sum = (np.asarray(bh, np.float32).reshape(U)
                + np.asarray(bs, np.float32).reshape(U)
                + np.asarray(bc, np.float32).reshape(U)).reshape(1, U)
    eye16 = np.eye(128, dtype=np.float32).astype(bf)
    eye32 = np.eye(128, dtype=np.float32)

    wc_t = np.tile(wc_row.astype(bf), (1, BL))
    in_maps = []
    for c in range(NCORES):
        sl = slice(c * BL, (c + 1) * BL)
        covA = np.empty((2, BL * S), dtype=bf)
        covA[0] = cov2[sl].reshape(-1).astype(bf)
        covA[1] = np.ones((BL * S,), dtype=bf)
        in_maps.append({
            "enc": enc_output[sl],
            "cov16": covA,
            "wc_t": wc_t,
            "cov32": cov2[sl],
            "mask": enc_padding_mask[sl],
            "decT": dec_hidden[sl].T.astype(bf),
            "wh": wh16,
            "ws": ws16,
            "wc8": wc8,
            "vw": vw_row,
            "bias_sum": bias_sum,
            "eye16": eye16,
            "eye32": eye32,
        })

    trace = os.environ.get("BASS_KERNEL_TRACE", "0") == "1"
    res = run_bass_kernel_spmd(nc, in_maps, core_ids=list(range(NCORES)),
                               trace=trace)
    if trace:
        LAST_EXEC_NS = res.exec_time_ns

    ctx = np.concatenate([r["out_ctx"] for r in res.results], axis=0)
    attn = np.concatenate([r["out_attn"] for r in res.results], axis=0)
    cov = np.concatenate([r["out_cov"] for r in res.results], axis=0)[..., None]
    return ctx.astype(np.float32), attn.astype(np.float32), cov.astype(np.float32)


# revision 20
# speedup vs baseline: 1.8301x; 1.1074x over previous
# Bahdanau attention with coverage — TRN2 Bass kernel, 8-way batch-parallel.
#
#   eh    = enc @ Wh + bh                      (B,S,U)
#   sh    = dec @ Ws + bs                      (B,U)
#   ch    = cov @ Wc + bc                      (B,S,U)
#   feat  = tanh(eh + sh[:,None,:] + ch)
#   score = (feat @ Vw + bv)[..., 0]           (B,S)   (bv shifts cancel in softmax)
#   attn  = softmax(score)*mask / sum(...)
#   coverage = attn[...,None] + cov
#   context  = einsum('bs,bsd->bd', attn, enc)
#
# Sharding: batch dim across 8 cores (8 batches/core). Weights replicated.
#
# Per-core data path (b = local batch 0..7, S=2048 split into 16 s-tiles of
# 128, D=U=512 split into 4 chunks of 128):
#   * enc[b] is DMA-loaded with an fp32->bf16 cast (SWDGE) into natural
#     layout [128(s%128), 16(s_tile), 512(d)], then transposed on the PE
#     (64 transposes/batch) into encT [128(d%128), 4(d_chk), 16(s_tile), 128].
#   * feat psum tile [128(s),512(u)] accumulates 4 matmuls
#     encT_tile.T @ Wh_chunk plus one K=2 matmul whose two contraction rows
#     add cov[s]*Wc[u] (coverage rank-1) and 1*colbias_b[u]
#     (colbias_b = bh+bs+bc+dec[b]@Ws).
#   * ACT tanh -> feat bf16; one DVE scalar_tensor_tensor op computes
#     feat*Vw with fused row-sum -> score column [128,1] per s-tile.
#   * Per-batch masked softmax on [128,16] columns (partition_all_reduce for
#     cross-partition max/sum).
#   * context: 16 accumulating matmuls attn_col.T @ enc_nat_tile -> [1,512].
#   * attn columns transposed back to rows on PE; coverage = attn + cov.

import os
import numpy as np
import ml_dtypes

B, S, D, U = 64, 2048, 512, 512
NCORES = 8
BL = B // NCORES          # batches per core
ST = S // 128             # s-tiles per batch
KC = D // 128             # contraction chunks

LAST_EXEC_NS = None       # set when BASS_KERNEL_TRACE=1
_NC_CACHE = None


def _build_program():
    import concourse.bacc as bacc
    import concourse.tile as tile
    from concourse import mybir, bass_isa

    f32 = mybir.dt.float32
    bf16 = mybir.dt.bfloat16
    Tanh = mybir.ActivationFunctionType.Tanh
    Exp = mybir.ActivationFunctionType.Exp
    X = mybir.AxisListType.X
    MUL = mybir.AluOpType.mult
    RMAX = bass_isa.ReduceOp.max
    RADD = bass_isa.ReduceOp.add

    nc = bacc.Bacc("TRN2", target_bir_lowering=False, debug=False,
                   num_devices=NCORES)

    # ---- DRAM I/O (per-core shard shapes) ----
    enc = nc.dram_tensor("enc", (BL, S, D), f32, kind="ExternalInput").ap()
    cov16 = nc.dram_tensor("cov16", (2, BL * S), bf16, kind="ExternalInput").ap()
    wc_t = nc.dram_tensor("wc_t", (1, BL * U), bf16, kind="ExternalInput").ap()
    cov32 = nc.dram_tensor("cov32", (BL, S), f32, kind="ExternalInput").ap()
    mask = nc.dram_tensor("mask", (BL, S), f32, kind="ExternalInput").ap()
    decT = nc.dram_tensor("decT", (D, BL), bf16, kind="ExternalInput").ap()
    wh = nc.dram_tensor("wh", (D, U), bf16, kind="ExternalInput").ap()
    ws = nc.dram_tensor("ws", (D, U), bf16, kind="ExternalInput").ap()
    wc8 = nc.dram_tensor("wc8", (BL, U), bf16, kind="ExternalInput").ap()
    vw = nc.dram_tensor("vw", (1, U), bf16, kind="ExternalInput").ap()
    bias_sum = nc.dram_tensor("bias_sum", (1, U), f32, kind="ExternalInput").ap()
    eye16 = nc.dram_tensor("eye16", (128, 128), bf16, kind="ExternalInput").ap()
    eye32 = nc.dram_tensor("eye32", (128, 128), f32, kind="ExternalInput").ap()

    out_ctx = nc.dram_tensor("out_ctx", (BL, D), f32, kind="ExternalOutput").ap()
    out_attn = nc.dram_tensor("out_attn", (BL, S), f32, kind="ExternalOutput").ap()
    out_cov = nc.dram_tensor("out_cov", (BL, S), f32, kind="ExternalOutput").ap()

    with tile.TileContext(nc) as tc:
        with (
            tc.tile_pool(name="const", bufs=1) as const,
            tc.tile_pool(name="encpool", bufs=2) as encpool,
            tc.tile_pool(name="feats", bufs=4) as feats,
            tc.tile_pool(name="cols", bufs=3) as cols,
            tc.tile_pool(name="rows", bufs=2) as rows,
            tc.tile_pool(name="ps_feat", bufs=4, space="PSUM") as ps_feat,
            tc.tile_pool(name="ps_tr", bufs=2, space="PSUM") as ps_tr,
            tc.tile_pool(name="ps_ctx", bufs=1, space="PSUM") as ps_ctx,
            tc.tile_pool(name="ps_small", bufs=1, space="PSUM") as ps_small,
        ):
            # ---------- constants ----------
            wh_sb = const.tile([128, KC, U], bf16)
            nc.sync.dma_start(out=wh_sb, in_=wh.rearrange("(k p) u -> p k u", p=128))
            ws_sb = const.tile([128, KC, U], bf16)
            nc.sync.dma_start(out=ws_sb, in_=ws.rearrange("(k p) u -> p k u", p=128))
            decT_sb = const.tile([128, KC, BL], bf16)
            nc.sync.dma_start(out=decT_sb, in_=decT.rearrange("(k p) b -> p k b", p=128))
            covA = const.tile([2, BL * S], bf16)
            nc.sync.dma_start(out=covA, in_=cov16)
            eye16_sb = const.tile([128, 128], bf16)
            nc.sync.dma_start(out=eye16_sb, in_=eye16)
            eye32_sb = const.tile([128, 128], f32)
            nc.sync.dma_start(out=eye32_sb, in_=eye32)
            vw_row = const.tile([1, U], bf16)
            nc.sync.dma_start(out=vw_row, in_=vw)
            vw_bc = const.tile([128, U], bf16)
            nc.gpsimd.partition_broadcast(vw_bc, vw_row)
            bias_row = const.tile([1, U], f32)
            nc.sync.dma_start(out=bias_row, in_=bias_sum)
            bias_bc = const.tile([BL, U], f32)
            nc.gpsimd.partition_broadcast(bias_bc, bias_row)

            # ---------- colbias = dec @ Ws + (bh+bs+bc) ----------
            ps_sh = ps_feat.tile([BL, U], f32, tag="psfeat")
            for k in range(KC):
                nc.tensor.matmul(ps_sh, decT_sb[:, k, :], ws_sb[:, k, :],
                                 start=(k == 0), stop=(k == KC - 1))
            colbias = const.tile([BL, U], bf16)
            nc.vector.tensor_add(colbias, ps_sh, bias_bc)
            # wccb[0, b*U:] = Wc row, wccb[1, b*U:] = colbias row b
            wccb = const.tile([2, BL * U], bf16)
            nc.sync.dma_start(out=wccb[0:1, :], in_=wc_t)
            nc.sync.dma_start(out=wccb[1:2, :].rearrange("o (b u) -> o b u", b=BL),
                              in_=colbias)

            # ---------- per-batch main loop ----------
            for b in range(BL):
                # natural-layout bf16 enc (cast during DMA, SWDGE)
                enc_nat = encpool.tile([128, ST, D], bf16, tag="enc_nat")
                enc_v = enc[b].rearrange("(t p) d -> p t d", p=128)
                H = ST // 2
                nc.gpsimd.dma_start(out=enc_nat[:, :H, :], in_=enc_v[:, :H, :])
                nc.gpsimd.dma_start(out=enc_nat[:, H:, :], in_=enc_v[:, H:, :])
                # PE transposes -> encT [128(d%128), KC, ST, 128(s)]
                # 4 d-chunk transposes share one psum tile -> single wide copy
                encT = encpool.tile([128, KC, ST, 128], bf16, tag="encT")
                for t in range(ST):
                    pt = ps_tr.tile([128, KC, 128], bf16, tag="ptr")
                    for k in range(KC):
                        nc.tensor.transpose(
                            pt[:, k, :], enc_nat[:, t, k * 128:(k + 1) * 128],
                            eye16_sb)
                    nc.vector.tensor_copy(out=encT[:, :, t, :], in_=pt)

                # mask columns for this batch: [128, ST]
                mrow = rows.tile([16, 128], f32, tag="mrow")
                nc.sync.dma_start(out=mrow,
                                  in_=mask[b].rearrange("(t x) -> t x", x=128))
                pmt = ps_small.tile([128, 16], f32, tag="small")
                nc.tensor.transpose(pmt, mrow, eye32_sb[0:16, 0:16])
                maskT = cols.tile([128, ST], f32, tag="maskT")
                nc.vector.tensor_copy(out=maskT, in_=pmt)

                score_cols = cols.tile([128, ST], f32, tag="score")
                TG = 4  # s-tiles per group: 4 psum tiles share Wh chunk loads
                for tg in range(ST // TG):
                    pfs = [ps_feat.tile([128, U], f32, tag="psfeat",
                                        name=f"pf_{b}_{tg}_{i}")
                           for i in range(TG)]
                    for k in range(KC):
                        for tt in range(TG):
                            t = tg * TG + tt
                            nc.tensor.matmul(pfs[tt], encT[:, k, t, :],
                                             wh_sb[:, k, :],
                                             start=(k == 0), stop=False)
                    for tt in range(TG):
                        t = tg * TG + tt
                        pf = pfs[tt]
                        nc.tensor.matmul(
                            pf,
                            covA[:, b * S + t * 128: b * S + (t + 1) * 128],
                            wccb[:, b * U:(b + 1) * U],
                            start=False, stop=True)
                        feat = feats.tile([128, U], bf16, tag="feat")
                        nc.scalar.activation(out=feat, in_=pf, func=Tanh)
                        # prod = feat * Vw  (row-sum fused into accum_out, DVE)
                        prod = feats.tile([128, U], bf16, tag="prod")
                        nc.vector.scalar_tensor_tensor(
                            out=prod, in0=feat, scalar=1.0, in1=vw_bc,
                            op0=MUL, op1=MUL,
                            accum_out=score_cols[:, t:t + 1])

                # ---------- masked softmax over s (columns layout) ----------
                rmax = cols.tile([128, 1], f32, tag="rmax")
                nc.vector.reduce_max(rmax, score_cols, axis=X)
                gmax = cols.tile([128, 1], f32, tag="gmax")
                nc.gpsimd.partition_all_reduce(gmax, rmax, channels=128,
                                               reduce_op=RMAX)
                negm = cols.tile([128, 1], f32, tag="negm")
                nc.scalar.mul(negm, gmax, -1.0)
                e_cols = cols.tile([128, ST], f32, tag="ecols")
                nc.scalar.activation(out=e_cols, in_=score_cols, func=Exp,
                                     bias=negm, scale=1.0)
                # emask = e*mask with fused row-sum
                emask = cols.tile([128, ST], f32, tag="emask")
                rsum = cols.tile([128, 1], f32, tag="rsum")
                nc.vector.scalar_tensor_tensor(
                    out=emask, in0=e_cols, scalar=1.0, in1=maskT,
                    op0=MUL, op1=MUL, accum_out=rsum)
                gsum = cols.tile([128, 1], f32, tag="gsum")
                nc.gpsimd.partition_all_reduce(gsum, rsum, channels=128,
                                               reduce_op=RADD)
                rcp = cols.tile([128, 1], f32, tag="rcp")
                nc.vector.reciprocal(rcp, gsum)
                attn_f = cols.tile([128, ST], f32, tag="attn_f")
                nc.vector.tensor_scalar_mul(attn_f, emask, rcp)
                attn_h = cols.tile([128, ST], bf16, tag="attn_h")
                nc.vector.tensor_scalar_mul(attn_h, emask, rcp)

                # ---------- context = attn @ enc ----------
                pctx = ps_ctx.tile([1, D], f32, tag="pctx")
                for t in range(ST):
                    nc.tensor.matmul(pctx, attn_h[:, t:t + 1], enc_nat[:, t, :],
                                     start=(t == 0), stop=(t == ST - 1))
                ctx_sb = rows.tile([1, D], f32, tag="ctx_sb")
                nc.scalar.copy(ctx_sb, pctx)
                nc.sync.dma_start(out=out_ctx[b:b + 1, :], in_=ctx_sb)

                # ---------- attn rows + coverage ----------
                pat = ps_small.tile([16, 128], f32, tag="small")
                nc.tensor.transpose(pat, attn_f, eye32_sb)
                attn_rows = rows.tile([16, 128], f32, tag="attn_rows")
                nc.scalar.copy(attn_rows, pat)
                nc.sync.dma_start(
                    out=out_attn[b].rearrange("(t x) -> t x", x=128), in_=attn_rows)
                cov_t = rows.tile([16, 128], f32, tag="cov_t")
                nc.sync.dma_start(out=cov_t,
                                  in_=cov32[b].rearrange("(t x) -> t x", x=128))
                cov_o = rows.tile([16, 128], f32, tag="cov_o")
                nc.vector.tensor_add(cov_o, pat, cov_t)
                nc.sync.dma_start(
                    out=out_cov[b].rearrange("(t x) -> t x", x=128), in_=cov_o)

    nc.finalize()
    return nc


def _get_program():
    global _NC_CACHE
    if _NC_CACHE is None:
        _NC_CACHE = _build_program()
    return _NC_CACHE


_LDW_PATCHED = False


def _maybe_patch_ldw_opt():
    # Optional experiment: let walrus optimize LDWEIGHTS (gated by env).
    global _LDW_PATCHED
    if _LDW_PATCHED or os.environ.get("BASS_LDW_OPT", "0") != "1":
        return
    import concourse.bass_utils as bu
    orig = bu.run_command

    def patched(argv, **kw):
        argv = ["--enable-ldw-opt=true" if a == "--enable-ldw-opt=false" else a
                for a in argv]
        return orig(argv, **kw)

    bu.run_command = patched
    _LDW_PATCHED = True


def kernel(dec_hidden, enc_output, enc_padding_mask, prev_coverage,
           Wh, bh, Ws, bs, Wc, bc, Vw, bv):
    global LAST_EXEC_NS
    _maybe_patch_ldw_opt()
    from concourse.bass_utils import run_bass_kernel_spmd

    nc = _get_program()
    bf = ml_dtypes.bfloat16

    dec_hidden = np.asarray(dec_hidden, dtype=np.float32)
    enc_output = np.asarray(enc_output, dtype=np.float32)
    enc_padding_mask = np.asarray(enc_padding_mask, dtype=np.float32)
    prev_coverage = np.asarray(prev_coverage, dtype=np.float32)

    cov2 = prev_coverage[..., 0]                      # (B, S)
    wh16 = np.asarray(Wh, np.float32).astype(bf)
    ws16 = np.asarray(Ws, np.float32).astype(bf)
    wc_row = np.asarray(Wc, np.float32).reshape(1, U)
    wc8 = np.ascontiguousarray(np.broadcast_to(wc_row, (BL, U))).astype(bf)
    vw_row = np.asarray(Vw, np.float32).reshape(1, U).astype(bf)
    bias_sum = (np.asarray(bh, np.float32).reshape(U)
                + np.asarray(bs, np.float32).reshape(U)
                + np.asarray(bc, np.float32).reshape(U)).reshape(1, U)
    eye16 = np.eye(128, dtype=np.float32).astype(bf)
    eye32 = np.eye(128, dtype=np.float32)

    wc_t = np.tile(wc_row.astype(bf), (1, BL))
    in_maps = []
    for c in range(NCORES):
        sl = slice(c * BL, (c + 1) * BL)
        covA = np.empty((2, BL * S), dtype=bf)
        covA[0] = cov2[sl].reshape(-1).astype(bf)
        covA[1] = np.ones((BL * S,), dtype=bf)
        in_maps.append({
            "enc": enc_output[sl],
            "cov16": covA,
            "wc_t": wc_t,
            "cov32": cov2[sl],
            "mask": enc_padding_mask[sl],
            "decT": dec_hidden[sl].T.astype(bf),
            "wh": wh16,
            "ws": ws16,
            "wc8": wc8,
            "vw": vw_row,
            "bias_sum": bias_sum,
            "eye16": eye16,
            "eye32": eye32,
        })

    trace = os.environ.get("BASS_KERNEL_TRACE", "0") == "1"
    res = run_bass_kernel_spmd(nc, in_maps, core_ids=list(range(NCORES)),
                               trace=trace)
    if trace:
        LAST_EXEC_NS = res.exec_time_ns

    ctx = np.concatenate([r["out_ctx"] for r in res.results], axis=0)
    attn = np.concatenate([r["out_attn"] for r in res.results], axis=0)
    cov = np.concatenate([r["out_cov"] for r in res.results], axis=0)[..., None]
    return ctx.astype(np.float32), attn.astype(np.float32), cov.astype(np.float32)
